# revision 20
# baseline (speedup 1.0000x reference)
"""Distributed Trainium2 Bass kernel for single-head attention with
softmax over the QUERY axis (faithful to the reference).

Reference math (per batch b):
    q = x @ Wq + bq          # [S, D]   S=4096, D=48
    k = x @ Wk + bk
    v = x @ Wv + bv
    s = (q @ k.T) / sqrt(D)  # [S_q, S_k]
    p = softmax(s, axis=QUERY)          # normalize each k-COLUMN over q
    out = p @ v              # [S_q, D]

Sharding: 8 cores = 4 batches x 2 query-halves. Core c handles batch
c//2, query rows [ (c%2)*2048, (c%2+1)*2048 ).

Layout: everything is computed TRANSPOSED on chip.
  - scores_T[k, q] tiles have k on partitions / q on the free axis, so
    the softmax denominator colsum[k] = sum_q exp(s[q,k]) is a free-axis
    reduction that the ScalarEngine emits for free via
    activation(Exp, accum_out=...).
  - The per-column normalization folds into V (V[k,:] /= colsum[k]).
  - colsum needs both query-halves: two 8 KB pairwise AllReduces,
    chunked over k so the first one hides under the exp phase.
  - Output is produced as out_T [48, 2048] and transposed on host.

Structure is ordered so the ScalarEngine (exp over 8.4M scores = the
roofline engine here) starts ~10us in and paces the whole middle of the
kernel: per 512-wide sequence chunk we emit [load xT chunk, K-proj,
V-proj, V-transposes, 4x (scores matmul + exp)], all overlapped by the
Tile scheduler. The attention matmuls are column-packed two-at-a-time
into the PE array (tile_position (0,0)/(0,64)) since their M=48 leaves
most of the array idle.

exp() runs without max-subtraction: scores*scale is N(0,~1/9), bounded
by ~|2.5| for these inputs, so exp stays well inside fp32 range
(softmax is shift-invariant, so the result matches the reference).
"""

import sys

for _p in ("/opt/trn_rl_repo",):
    if _p not in sys.path:
        sys.path.insert(0, _p)

import numpy as np
import ml_dtypes

import concourse.bass as bass
import concourse.tile as tile
from concourse import bacc, mybir
from concourse.bass_utils import run_bass_kernel_spmd
from concourse.masks import make_identity

N_CORES = 8
B = 4
S = 4096
DIM = 768
D = 48
SH = S // 2          # query rows per core
P = 128
NK = S // P          # 32 k-tiles
NC = DIM // P        # 6 contraction tiles for projections
QF = 512             # matmul moving free dim
NQC = SH // QF       # 4 q-chunks per core
NSC = S // QF        # 8 s-chunks for K/V projections
KPC = QF // P        # 4 k-tiles per s-chunk
SCALE = 1.0 / np.sqrt(np.float32(D))
# colsum AllReduce group boundaries (in k-tiles). Non-uniform: the first
# AR absorbs the ~34us collective warmup while exp still runs; later ARs
# pipeline at ~8.5us marginal, so only the last small group's AR+attn is
# exposed after the exp phase ends.
AR_BOUNDS = [16, 24, 28, 32]

BF16 = mybir.dt.bfloat16
F32 = mybir.dt.float32


def _build():
    nc = bacc.Bacc(
        "TRN2",
        target_bir_lowering=False,
        debug=False,
        num_devices=N_CORES,
    )

    xt_d = nc.dram_tensor("xt", [DIM, S], BF16, kind="ExternalInput")
    xtq_d = nc.dram_tensor("xtq", [DIM, SH], BF16, kind="ExternalInput")
    wq_d = nc.dram_tensor("wq", [DIM, D], BF16, kind="ExternalInput")
    wkv_d = nc.dram_tensor("wkv", [DIM, 112], BF16, kind="ExternalInput")
    bq_d = nc.dram_tensor("bq", [D, 1], F32, kind="ExternalInput")
    bkv_d = nc.dram_tensor("bkv", [112, 1], F32, kind="ExternalInput")
    out_d = nc.dram_tensor("out", [D, SH], F32, kind="ExternalOutput")

    with tile.TileContext(nc) as tc:
        with (
            tc.tile_pool(name="consts", bufs=1) as consts,
            tc.tile_pool(name="big", bufs=1) as big,
            tc.tile_pool(name="xtp", bufs=3) as xtp,
            tc.tile_pool(name="ps", bufs=2, space="PSUM") as ps,
            tc.tile_pool(name="dram", bufs=1, space="DRAM") as dram,
        ):
            # ---- constants; Q-path DMAs first so exp starts early ---------
            wq_sb = consts.tile([P, NC, D], BF16, tag="wq")
            nc.sync.dma_start(out=wq_sb, in_=wq_d[:, :].rearrange("(i p) d -> p i d", p=P))
            bq_sb = consts.tile([D, 1], F32, tag="bq")
            nc.sync.dma_start(out=bq_sb, in_=bq_d[:, :])
            wkv_sb = consts.tile([P, NC, 112], BF16, tag="wkv")
            nc.sync.dma_start(out=wkv_sb, in_=wkv_d[:, :].rearrange("(i p) d -> p i d", p=P))
            bkv_sb = consts.tile([112, 1], F32, tag="bkv")
            nc.sync.dma_start(out=bkv_sb, in_=bkv_d[:, :])
            ident = consts.tile([P, P], BF16, tag="ident")
            make_identity(nc, ident)

            # ---- persistent SBUF tensors ----------------------------------
            kT_sb = big.tile([D, S], BF16, tag="kT")      # K^T  [48, 4096]
            vT_sb = big.tile([112, S], BF16, tag="vT")    # V^T @64-111, shifted->0-47
            qT_sb = big.tile([D, SH], BF16, tag="qT")     # Q^T  [48, 2048]
            v_sb = big.tile([P, NK, D], BF16, tag="v")    # V    [k, d] tiles
            vs_sb = big.tile([P, NK, D], BF16, tag="vs")  # V / colsum
            e_sb = big.tile([P, NK, SH], BF16, tag="e")   # E_T  [k, q] tiles
            colsum = big.tile([P, NK], F32, tag="colsum")
            recip = big.tile([P, NK], F32, tag="recip")
            cs_all = big.tile([P, NK], F32, tag="cs_all")
            out_sb = big.tile([D, SH], F32, tag="out")

            # ---- Q^T projection (own query half) --------------------------
            for qc in range(NQC):
                xq_t = xtp.tile([P, NC, QF], BF16, tag="xt")
                nc.sync.dma_start(
                    out=xq_t,
                    in_=xtq_d[:, qc * QF:(qc + 1) * QF].rearrange(
                        "(i p) f -> p i f", p=P
                    ),
                )
                pq = ps.tile([D, QF], F32, tag="ps")
                for ci in range(NC):
                    nc.tensor.matmul(
                        pq, wq_sb[:, ci, :], xq_t[:, ci, :],
                        start=(ci == 0), stop=(ci == NC - 1),
                    )
                nc.vector.tensor_scalar(
                    out=qT_sb[:, qc * QF:(qc + 1) * QF],
                    in0=pq, scalar1=bq_sb, scalar2=None,
                    op0=mybir.AluOpType.add,
                )

            # ---- interleaved: per s-chunk K/V proj + transposes + scores --
            ar_tiles = []
            for sc in range(NSC):
                sl = slice(sc * QF, (sc + 1) * QF)
                xt_t = xtp.tile([P, NC, QF], BF16, tag="xt")
                nc.sync.dma_start(
                    out=xt_t,
                    in_=xt_d[:, sl].rearrange("(i p) f -> p i f", p=P),
                )
                # fused K|V projection: one M=96 accumulation per chunk
                pkv = ps.tile([112, QF], F32, tag="ps")
                for ci in range(NC):
                    nc.tensor.matmul(
                        pkv, wkv_sb[:, ci, :], xt_t[:, ci, :],
                        start=(ci == 0), stop=(ci == NC - 1),
                    )
                nc.vector.tensor_scalar(
                    out=kT_sb[:, sl], in0=pkv[0:D, :], scalar1=bkv_sb[0:D, :],
                    scalar2=None, op0=mybir.AluOpType.add,
                )
                nc.vector.tensor_scalar(
                    out=vT_sb[64:64 + D, sl], in0=pkv[64:64 + D, :],
                    scalar1=bkv_sb[64:64 + D, :], scalar2=None,
                    op0=mybir.AluOpType.add,
                )
                # shift V^T down to partitions 0-47 (PE transpose wants a
                # {0,32,64}-aligned base), then transpose to V[k, d] tiles
                nc.sync.dma_start(out=vT_sb[0:D, sl], in_=vT_sb[64:64 + D, sl])
                for j in range(KPC):
                    kt = sc * KPC + j
                    pt = ps.tile([P, D], BF16, tag="ps")
                    nc.tensor.transpose(
                        pt, vT_sb[0:D, kt * P:(kt + 1) * P], ident[0:D, 0:D]
                    )
                    nc.vector.tensor_copy(out=v_sb[:, kt, :], in_=pt)
                # scores + exp for this chunk's k-tiles
                for j in range(KPC):
                    kt = sc * KPC + j
                    sct = ps.tile([P, NQC, QF], F32, tag="ps")
                    for qc in range(NQC):
                        nc.tensor.matmul(
                            sct[:, qc, :],
                            kT_sb[:, kt * P:(kt + 1) * P],
                            qT_sb[:, qc * QF:(qc + 1) * QF],
                            start=True, stop=True,
                        )
                    nc.scalar.activation(
                        out=e_sb[:, kt, :],
                        in_=sct[:, :, :],
                        func=mybir.ActivationFunctionType.Exp,
                        scale=float(SCALE),
                        accum_out=colsum[:, kt:kt + 1],
                    )
                # issue the group's colsum AllReduce as soon as its exps done
                kt_done = (sc + 1) * KPC
                if kt_done in AR_BOUNDS:
                    g = AR_BOUNDS.index(kt_done)
                    kt_lo = 0 if g == 0 else AR_BOUNDS[g - 1]
                    gsl = slice(kt_lo, kt_done)
                    gn = kt_done - kt_lo
                    cs_in = dram.tile([P, gn], F32, tag=f"cs_in{g}")
                    cs_out = dram.tile([P, gn], F32, tag=f"cs_out{g}")
                    nc.sync.dma_start(out=cs_in, in_=colsum[:, gsl])
                    nc.gpsimd.collective_compute(
                        "AllReduce",
                        mybir.AluOpType.add,
                        replica_groups=[[0, 1], [2, 3], [4, 5], [6, 7]],
                        ins=[cs_in.opt()],
                        outs=[cs_out.opt()],
                    )
                    nc.sync.dma_start(out=cs_all[:, gsl], in_=cs_out)
                    nc.vector.reciprocal(out=recip[:, gsl], in_=cs_all[:, gsl])
                    for kt in range(kt_lo, kt_done):
                        nc.vector.tensor_scalar(
                            out=vs_sb[:, kt, :],
                            in0=v_sb[:, kt, :],
                            scalar1=recip[:, kt:kt + 1], scalar2=None,
                            op0=mybir.AluOpType.mult,
                        )

            # ---- attention matmuls, column-packed 2 k-tiles at a time -----
            po = ps.tile([P, NQC, QF], F32, tag="ps")
            npair = NK // 2
            for p_i in range(npair):
                ktA, ktB = 2 * p_i, 2 * p_i + 1
                for qc in range(NQC):
                    qsl = slice(qc * QF, (qc + 1) * QF)
                    nc.tensor.matmul(
                        po[0:D, qc, :],
                        vs_sb[:, ktA, :],
                        e_sb[:, ktA, qsl],
                        start=(p_i == 0), stop=(p_i == npair - 1),
                        tile_position=(0, 0), skip_group_check=True,
                    )
                    nc.tensor.matmul(
                        po[64:64 + D, qc, :],
                        vs_sb[:, ktB, :],
                        e_sb[:, ktB, qsl],
                        start=(p_i == 0), stop=(p_i == npair - 1),
                        tile_position=(0, 64), skip_group_check=True,
                    )
            nc.vector.tensor_copy(out=out_sb, in_=po[0:D, :, :])
            nc.vector.tensor_add(
                out=out_sb,
                in0=out_sb,
                in1=po[64:64 + D, :, :],
            )
            nc.sync.dma_start(out=out_d[:, :], in_=out_sb)

    nc.compile()
    return nc


_NC_CACHE = None


def _get_nc():
    global _NC_CACHE
    if _NC_CACHE is None:
        _NC_CACHE = _build()
    return _NC_CACHE


def kernel(x, Wq, bq, Wk, bk, Wv, bv):
    x = np.asarray(x, np.float32)
    bf = ml_dtypes.bfloat16
    wkv = np.zeros((DIM, 112), np.float32)
    wkv[:, 0:D] = np.asarray(Wk, np.float32)
    wkv[:, 64:64 + D] = np.asarray(Wv, np.float32)
    bkv = np.zeros((112,), np.float32)
    bkv[0:D] = np.asarray(bk, np.float32).ravel()
    bkv[64:64 + D] = np.asarray(bv, np.float32).ravel()
    w_bf = {
        "wq": np.ascontiguousarray(np.asarray(Wq, np.float32)).astype(bf),
        "wkv": np.ascontiguousarray(wkv).astype(bf),
    }
    b_f32 = {
        "bq": np.ascontiguousarray(np.asarray(bq, np.float32)).reshape(D, 1),
        "bkv": np.ascontiguousarray(bkv).reshape(112, 1),
    }

    in_maps = []
    for core in range(N_CORES):
        b_idx, h = divmod(core, 2)
        xt = np.ascontiguousarray(x[b_idx].T).astype(bf)          # [768, 4096]
        xtq = np.ascontiguousarray(xt[:, h * SH:(h + 1) * SH])    # [768, 2048]
        in_maps.append({"xt": xt, "xtq": xtq, **w_bf, **b_f32})

    res = run_bass_kernel_spmd(
        _get_nc(), in_maps, core_ids=list(range(N_CORES)), trace=False
    )

    out = np.empty((B, S, D), np.float32)
    for core in range(N_CORES):
        b_idx, h = divmod(core, 2)
        out[b_idx, h * SH:(h + 1) * SH, :] = res.results[core]["out"].T
    return out


# revision 21
# speedup vs baseline: 1.1492x; 1.1492x over previous
"""Distributed Trainium2 Bass kernel for single-head attention with
softmax over the QUERY axis (faithful to the reference).

Reference math (per batch b):
    q = x @ Wq + bq          # [S, D]   S=4096, D=48
    k = x @ Wk + bk
    v = x @ Wv + bv
    s = (q @ k.T) / sqrt(D)  # [S_q, S_k]
    p = softmax(s, axis=QUERY)          # normalize each k-COLUMN over q
    out = p @ v              # [S_q, D]

Sharding: 8 cores = 4 batches x 2 query-halves. Core c handles batch
c//2, query rows [ (c%2)*2048, (c%2+1)*2048 ).

Layout: everything is computed TRANSPOSED on chip.
  - scores_T[k, q] tiles have k on partitions / q on the free axis, so
    the softmax denominator colsum[k] = sum_q exp(s[q,k]) is a free-axis
    reduction that the ScalarEngine emits for free via
    activation(Exp, accum_out=...).
  - The per-column normalization folds into V (V[k,:] /= colsum[k]).
  - colsum needs both query-halves: two 8 KB pairwise AllReduces,
    chunked over k so the first one hides under the exp phase.
  - Output is produced as out_T [48, 2048] and transposed on host.

Structure is ordered so the ScalarEngine (exp over 8.4M scores = the
roofline engine here) starts ~10us in and paces the whole middle of the
kernel: per 512-wide sequence chunk we emit [load xT chunk, K-proj,
V-proj, V-transposes, 4x (scores matmul + exp)], all overlapped by the
Tile scheduler. The attention matmuls are column-packed two-at-a-time
into the PE array (tile_position (0,0)/(0,64)) since their M=48 leaves
most of the array idle.

exp() runs without max-subtraction: scores*scale is N(0,~1/9), bounded
by ~|2.5| for these inputs, so exp stays well inside fp32 range
(softmax is shift-invariant, so the result matches the reference).
"""

import sys

for _p in ("/opt/trn_rl_repo",):
    if _p not in sys.path:
        sys.path.insert(0, _p)

import numpy as np
import ml_dtypes

import concourse.bass as bass
import concourse.tile as tile
from concourse import bacc, mybir
from concourse.bass_utils import run_bass_kernel_spmd
from concourse.masks import make_identity

N_CORES = 8
B = 4
S = 4096
DIM = 768
D = 48
SH = S // 2          # query rows per core
P = 128
NK = S // P          # 32 k-tiles
NC = DIM // P        # 6 contraction tiles for projections
QF = 512             # matmul moving free dim
NQC = SH // QF       # 4 q-chunks per core
NSC = S // QF        # 8 s-chunks for K/V projections
KPC = QF // P        # 4 k-tiles per s-chunk
SCALE = 1.0 / np.sqrt(np.float32(D))
# colsum AllReduce group boundaries (in k-tiles). Non-uniform: the first
# AR absorbs the ~34us collective warmup while exp still runs; later ARs
# pipeline at ~8.5us marginal, so only the last small group's AR+attn is
# exposed after the exp phase ends.
AR_BOUNDS = [16, 24, 32]

BF16 = mybir.dt.bfloat16
F32 = mybir.dt.float32


def _build():
    nc = bacc.Bacc(
        "TRN2",
        target_bir_lowering=False,
        debug=False,
        num_devices=N_CORES,
    )

    xt_d = nc.dram_tensor("xt", [DIM, S], BF16, kind="ExternalInput")
    xtq_d = nc.dram_tensor("xtq", [DIM, SH], BF16, kind="ExternalInput")
    wq_d = nc.dram_tensor("wq", [DIM, D], BF16, kind="ExternalInput")
    wk_d = nc.dram_tensor("wk", [DIM, D], BF16, kind="ExternalInput")
    wv_d = nc.dram_tensor("wv", [DIM, D], BF16, kind="ExternalInput")
    bq_d = nc.dram_tensor("bq", [D, 1], F32, kind="ExternalInput")
    bk_d = nc.dram_tensor("bk", [D, 1], F32, kind="ExternalInput")
    bv_d = nc.dram_tensor("bv", [D, 1], F32, kind="ExternalInput")
    out_d = nc.dram_tensor("out", [D, SH], F32, kind="ExternalOutput")

    with tile.TileContext(nc) as tc:
        with (
            tc.tile_pool(name="consts", bufs=1) as consts,
            tc.tile_pool(name="big", bufs=1) as big,
            tc.tile_pool(name="xtp", bufs=3) as xtp,
            tc.tile_pool(name="ps", bufs=2, space="PSUM") as ps,
            tc.tile_pool(name="dram", bufs=1, space="DRAM") as dram,
        ):
            # ---- constants; Q-path DMAs first so exp starts early ---------
            wq_sb = consts.tile([P, NC, D], BF16, tag="wq")
            nc.sync.dma_start(out=wq_sb, in_=wq_d[:, :].rearrange("(i p) d -> p i d", p=P))
            bq_sb = consts.tile([D, 1], F32, tag="bq")
            nc.sync.dma_start(out=bq_sb, in_=bq_d[:, :])
            wk_sb = consts.tile([P, NC, D], BF16, tag="wk")
            nc.sync.dma_start(out=wk_sb, in_=wk_d[:, :].rearrange("(i p) d -> p i d", p=P))
            bk_sb = consts.tile([D, 1], F32, tag="bk")
            nc.sync.dma_start(out=bk_sb, in_=bk_d[:, :])
            wv_sb = consts.tile([P, NC, D], BF16, tag="wv")
            nc.sync.dma_start(out=wv_sb, in_=wv_d[:, :].rearrange("(i p) d -> p i d", p=P))
            bv_sb = consts.tile([D, 1], F32, tag="bv")
            nc.sync.dma_start(out=bv_sb, in_=bv_d[:, :])
            ident = consts.tile([P, P], BF16, tag="ident")
            make_identity(nc, ident)

            # ---- persistent SBUF tensors ----------------------------------
            kT_sb = big.tile([D, S], BF16, tag="kT")      # K^T  [48, 4096]
            vT_sb = big.tile([D, S], BF16, tag="vT")      # V^T  [48, 4096]
            qT_sb = big.tile([D, SH], BF16, tag="qT")     # Q^T  [48, 2048]
            v_sb = big.tile([P, NK, D], BF16, tag="v")    # V    [k, d] tiles
            vs_sb = big.tile([P, NK, D], BF16, tag="vs")  # V / colsum
            e_sb = big.tile([P, NK, SH], BF16, tag="e")   # E_T  [k, q] tiles
            colsum = big.tile([P, NK], F32, tag="colsum")
            recip = big.tile([P, NK], F32, tag="recip")
            cs_all = big.tile([P, NK], F32, tag="cs_all")
            out_sb = big.tile([D, SH], F32, tag="out")

            # ---- Q^T projection (own query half) --------------------------
            for qc in range(NQC):
                xq_t = xtp.tile([P, NC, QF], BF16, tag="xt")
                nc.sync.dma_start(
                    out=xq_t,
                    in_=xtq_d[:, qc * QF:(qc + 1) * QF].rearrange(
                        "(i p) f -> p i f", p=P
                    ),
                )
                pq = ps.tile([D, QF], F32, tag="ps")
                for ci in range(NC):
                    nc.tensor.matmul(
                        pq, wq_sb[:, ci, :], xq_t[:, ci, :],
                        start=(ci == 0), stop=(ci == NC - 1),
                    )
                nc.vector.tensor_scalar(
                    out=qT_sb[:, qc * QF:(qc + 1) * QF],
                    in0=pq, scalar1=bq_sb, scalar2=None,
                    op0=mybir.AluOpType.add,
                )

            # ---- software-pipelined: proj(sc+1) overlaps scores/exp(sc) --
            def kv_chunk(sc):
                sl = slice(sc * QF, (sc + 1) * QF)
                xt_t = xtp.tile([P, NC, QF], BF16, tag="xt")
                nc.sync.dma_start(
                    out=xt_t,
                    in_=xt_d[:, sl].rearrange("(i p) f -> p i f", p=P),
                )
                pk = ps.tile([D, QF], F32, tag="ps")
                for ci in range(NC):
                    nc.tensor.matmul(
                        pk, wk_sb[:, ci, :], xt_t[:, ci, :],
                        start=(ci == 0), stop=(ci == NC - 1),
                    )
                nc.vector.tensor_scalar(
                    out=kT_sb[:, sl], in0=pk, scalar1=bk_sb, scalar2=None,
                    op0=mybir.AluOpType.add,
                )
                pv = ps.tile([D, QF], F32, tag="ps")
                for ci in range(NC):
                    nc.tensor.matmul(
                        pv, wv_sb[:, ci, :], xt_t[:, ci, :],
                        start=(ci == 0), stop=(ci == NC - 1),
                    )
                nc.vector.tensor_scalar(
                    out=vT_sb[:, sl], in0=pv, scalar1=bv_sb, scalar2=None,
                    op0=mybir.AluOpType.add,
                )
                for j in range(KPC):
                    kt = sc * KPC + j
                    pt = ps.tile([P, D], BF16, tag="ps")
                    nc.tensor.transpose(
                        pt, vT_sb[:, kt * P:(kt + 1) * P], ident[0:D, 0:D]
                    )
                    nc.vector.tensor_copy(out=v_sb[:, kt, :], in_=pt)

            kv_chunk(0)
            for sc in range(NSC):
                for j in range(KPC):
                    kt = sc * KPC + j
                    sct = ps.tile([P, NQC, QF], F32, tag="ps")
                    for qc in range(NQC):
                        nc.tensor.matmul(
                            sct[:, qc, :],
                            kT_sb[:, kt * P:(kt + 1) * P],
                            qT_sb[:, qc * QF:(qc + 1) * QF],
                            start=True, stop=True,
                        )
                    nc.scalar.activation(
                        out=e_sb[:, kt, :],
                        in_=sct[:, :, :],
                        func=mybir.ActivationFunctionType.Exp,
                        scale=float(SCALE),
                        accum_out=colsum[:, kt:kt + 1],
                    )
                if sc + 1 < NSC:
                    kv_chunk(sc + 1)
                # issue the group's colsum AllReduce as soon as its exps done
                kt_done = (sc + 1) * KPC
                if kt_done in AR_BOUNDS:
                    g = AR_BOUNDS.index(kt_done)
                    kt_lo = 0 if g == 0 else AR_BOUNDS[g - 1]
                    gsl = slice(kt_lo, kt_done)
                    gn = kt_done - kt_lo
                    cs_in = dram.tile([P, gn], F32, tag=f"cs_in{g}")
                    cs_out = dram.tile([P, gn], F32, tag=f"cs_out{g}")
                    nc.sync.dma_start(out=cs_in, in_=colsum[:, gsl])
                    nc.gpsimd.collective_compute(
                        "AllReduce",
                        mybir.AluOpType.add,
                        replica_groups=[[0, 1], [2, 3], [4, 5], [6, 7]],
                        ins=[cs_in.opt()],
                        outs=[cs_out.opt()],
                    )
                    nc.sync.dma_start(out=cs_all[:, gsl], in_=cs_out)
                    nc.vector.reciprocal(out=recip[:, gsl], in_=cs_all[:, gsl])
                    for kt in range(kt_lo, kt_done):
                        nc.vector.tensor_scalar(
                            out=vs_sb[:, kt, :],
                            in0=v_sb[:, kt, :],
                            scalar1=recip[:, kt:kt + 1], scalar2=None,
                            op0=mybir.AluOpType.mult,
                        )

            # ---- attention matmuls, column-packed 2 k-tiles at a time -----
            po = ps.tile([P, NQC, QF], F32, tag="ps")
            npair = NK // 2
            for p_i in range(npair):
                ktA, ktB = 2 * p_i, 2 * p_i + 1
                for qc in range(NQC):
                    qsl = slice(qc * QF, (qc + 1) * QF)
                    nc.tensor.matmul(
                        po[0:D, qc, :],
                        vs_sb[:, ktA, :],
                        e_sb[:, ktA, qsl],
                        start=(p_i == 0), stop=(p_i == npair - 1),
                        tile_position=(0, 0), skip_group_check=True,
                    )
                    nc.tensor.matmul(
                        po[64:64 + D, qc, :],
                        vs_sb[:, ktB, :],
                        e_sb[:, ktB, qsl],
                        start=(p_i == 0), stop=(p_i == npair - 1),
                        tile_position=(0, 64), skip_group_check=True,
                    )
            nc.vector.tensor_copy(out=out_sb, in_=po[0:D, :, :])
            nc.vector.tensor_add(
                out=out_sb,
                in0=out_sb,
                in1=po[64:64 + D, :, :],
            )
            nc.sync.dma_start(out=out_d[:, :], in_=out_sb)

    nc.compile()
    return nc


_NC_CACHE = None


def _get_nc():
    global _NC_CACHE
    if _NC_CACHE is None:
        _NC_CACHE = _build()
    return _NC_CACHE


def kernel(x, Wq, bq, Wk, bk, Wv, bv):
    x = np.asarray(x, np.float32)
    bf = ml_dtypes.bfloat16
    w_bf = {
        "wq": np.ascontiguousarray(np.asarray(Wq, np.float32)).astype(bf),
        "wk": np.ascontiguousarray(np.asarray(Wk, np.float32)).astype(bf),
        "wv": np.ascontiguousarray(np.asarray(Wv, np.float32)).astype(bf),
    }
    b_f32 = {
        "bq": np.ascontiguousarray(np.asarray(bq, np.float32)).reshape(D, 1),
        "bk": np.ascontiguousarray(np.asarray(bk, np.float32)).reshape(D, 1),
        "bv": np.ascontiguousarray(np.asarray(bv, np.float32)).reshape(D, 1),
    }

    in_maps = []
    for core in range(N_CORES):
        b_idx, h = divmod(core, 2)
        xt = np.ascontiguousarray(x[b_idx].T).astype(bf)          # [768, 4096]
        xtq = np.ascontiguousarray(xt[:, h * SH:(h + 1) * SH])    # [768, 2048]
        in_maps.append({"xt": xt, "xtq": xtq, **w_bf, **b_f32})

    res = run_bass_kernel_spmd(
        _get_nc(), in_maps, core_ids=list(range(N_CORES)), trace=False
    )

    out = np.empty((B, S, D), np.float32)
    for core in range(N_CORES):
        b_idx, h = divmod(core, 2)
        out[b_idx, h * SH:(h + 1) * SH, :] = res.results[core]["out"].T
    return out


# revision 22
# speedup vs baseline: 1.2883x; 1.1211x over previous
"""Distributed Trainium2 Bass kernel for single-head attention with
softmax over the QUERY axis (faithful to the reference).

Reference math (per batch b):
    q = x @ Wq + bq          # [S, D]   S=4096, D=48
    k = x @ Wk + bk
    v = x @ Wv + bv
    s = (q @ k.T) / sqrt(D)  # [S_q, S_k]
    p = softmax(s, axis=QUERY)          # normalize each k-COLUMN over q
    out = p @ v              # [S_q, D]

Sharding: 8 cores = 4 batches x 2 query-halves. Core c handles batch
c//2, query rows [ (c%2)*2048, (c%2+1)*2048 ).

Layout: everything is computed TRANSPOSED on chip.
  - scores_T[k, q] tiles have k on partitions / q on the free axis, so
    the softmax denominator colsum[k] = sum_q exp(s[q,k]) is a free-axis
    reduction that the ScalarEngine emits for free via
    activation(Exp, accum_out=...).
  - The per-column normalization folds into V (V[k,:] /= colsum[k]).
  - colsum needs both query-halves: two 8 KB pairwise AllReduces,
    chunked over k so the first one hides under the exp phase.
  - Output is produced as out_T [48, 2048] and transposed on host.

Structure is ordered so the ScalarEngine (exp over 8.4M scores = the
roofline engine here) starts ~10us in and paces the whole middle of the
kernel: per 512-wide sequence chunk we emit [load xT chunk, K-proj,
V-proj, V-transposes, 4x (scores matmul + exp)], all overlapped by the
Tile scheduler. The attention matmuls are column-packed two-at-a-time
into the PE array (tile_position (0,0)/(0,64)) since their M=48 leaves
most of the array idle.

exp() runs without max-subtraction: scores*scale is N(0,~1/9), bounded
by ~|2.5| for these inputs, so exp stays well inside fp32 range
(softmax is shift-invariant, so the result matches the reference).
"""

import sys

for _p in ("/opt/trn_rl_repo",):
    if _p not in sys.path:
        sys.path.insert(0, _p)

import numpy as np
import ml_dtypes

import concourse.bass as bass
import concourse.tile as tile
from concourse import bacc, mybir
from concourse.bass_utils import run_bass_kernel_spmd
from concourse.masks import make_identity

N_CORES = 8
B = 4
S = 4096
DIM = 768
D = 48
SH = S // 2          # query rows per core
P = 128
NK = S // P          # 32 k-tiles
NC = DIM // P        # 6 contraction tiles for projections
QF = 512             # matmul moving free dim
NQC = SH // QF       # 4 q-chunks per core
NSC = S // QF        # 8 s-chunks for K/V projections
KPC = QF // P        # 4 k-tiles per s-chunk
SCALE = 1.0 / np.sqrt(np.float32(D))
# colsum AllReduce group boundaries (in k-tiles). Non-uniform: the first
# AR absorbs the ~34us collective warmup while exp still runs; later ARs
# pipeline at ~8.5us marginal, so only the last small group's AR+attn is
# exposed after the exp phase ends.
AR_BOUNDS = [16, 24, 32]

BF16 = mybir.dt.bfloat16
F32 = mybir.dt.float32


def _build():
    nc = bacc.Bacc(
        "TRN2",
        target_bir_lowering=False,
        debug=False,
        num_devices=N_CORES,
    )

    xt_d = nc.dram_tensor("xt", [DIM, S], BF16, kind="ExternalInput")
    xtq_d = nc.dram_tensor("xtq", [DIM, SH], BF16, kind="ExternalInput")
    wq_d = nc.dram_tensor("wq", [DIM, D], BF16, kind="ExternalInput")
    wk_d = nc.dram_tensor("wk", [DIM, D], BF16, kind="ExternalInput")
    wv_d = nc.dram_tensor("wv", [DIM, D], BF16, kind="ExternalInput")
    bq_d = nc.dram_tensor("bq", [D, 1], F32, kind="ExternalInput")
    bk_d = nc.dram_tensor("bk", [D, 1], F32, kind="ExternalInput")
    bv_d = nc.dram_tensor("bv", [D, 1], F32, kind="ExternalInput")
    out_d = nc.dram_tensor("out", [D, SH], F32, kind="ExternalOutput")

    with tile.TileContext(nc) as tc:
        with (
            tc.tile_pool(name="consts", bufs=1) as consts,
            tc.tile_pool(name="big", bufs=1) as big,
            tc.tile_pool(name="xtp", bufs=3) as xtp,
            tc.tile_pool(name="ps", bufs=2, space="PSUM") as ps,
            tc.tile_pool(name="dram", bufs=1, space="DRAM") as dram,
        ):
            # ---- constants; Q-path DMAs first so exp starts early ---------
            wq_sb = consts.tile([P, NC, D], BF16, tag="wq")
            nc.sync.dma_start(out=wq_sb, in_=wq_d[:, :].rearrange("(i p) d -> p i d", p=P))
            bq_sb = consts.tile([D, 1], F32, tag="bq")
            nc.sync.dma_start(out=bq_sb, in_=bq_d[:, :])
            wk_sb = consts.tile([P, NC, D], BF16, tag="wk")
            nc.sync.dma_start(out=wk_sb, in_=wk_d[:, :].rearrange("(i p) d -> p i d", p=P))
            bk_sb = consts.tile([D, 1], F32, tag="bk")
            nc.sync.dma_start(out=bk_sb, in_=bk_d[:, :])
            wv_sb = consts.tile([P, NC, D], BF16, tag="wv")
            nc.sync.dma_start(out=wv_sb, in_=wv_d[:, :].rearrange("(i p) d -> p i d", p=P))
            bv_sb = consts.tile([D, 1], F32, tag="bv")
            nc.sync.dma_start(out=bv_sb, in_=bv_d[:, :])
            ident = consts.tile([P, P], BF16, tag="ident")
            make_identity(nc, ident)

            # ---- persistent SBUF tensors ----------------------------------
            kT_sb = big.tile([D, S], BF16, tag="kT")      # K^T  [48, 4096]
            vT_sb = big.tile([D, S], BF16, tag="vT")      # V^T  [48, 4096]
            qT_sb = big.tile([D, SH], BF16, tag="qT")     # Q^T  [48, 2048]
            v_sb = big.tile([P, NK, D], BF16, tag="v")    # V    [k, d] tiles
            vs_sb = big.tile([P, NK, D], BF16, tag="vs")  # V / colsum
            e_sb = big.tile([P, NK, SH], BF16, tag="e")   # E_T  [k, q] tiles
            colsum = big.tile([P, NK], F32, tag="colsum")
            recip = big.tile([P, NK], F32, tag="recip")
            cs_all = big.tile([P, NK], F32, tag="cs_all")
            out_sb = big.tile([D, SH], F32, tag="out")

            # ---- Q^T projection (own query half) --------------------------
            for qc in range(NQC):
                xq_t = xtp.tile([P, NC, QF], BF16, tag="xt")
                nc.sync.dma_start(
                    out=xq_t,
                    in_=xtq_d[:, qc * QF:(qc + 1) * QF].rearrange(
                        "(i p) f -> p i f", p=P
                    ),
                )
                pq = ps.tile([D, QF], F32, tag="ps")
                for ci in range(NC):
                    nc.tensor.matmul(
                        pq, wq_sb[:, ci, :], xq_t[:, ci, :],
                        start=(ci == 0), stop=(ci == NC - 1),
                    )
                nc.vector.tensor_scalar(
                    out=qT_sb[:, qc * QF:(qc + 1) * QF],
                    in0=pq, scalar1=bq_sb, scalar2=None,
                    op0=mybir.AluOpType.add,
                )

            # ---- software-pipelined: proj(sc+1) overlaps scores/exp(sc) --
            def kv_chunk(sc):
                sl = slice(sc * QF, (sc + 1) * QF)
                xt_t = xtp.tile([P, NC, QF], BF16, tag="xt")
                nc.sync.dma_start(
                    out=xt_t,
                    in_=xt_d[:, sl].rearrange("(i p) f -> p i f", p=P),
                )
                pk = ps.tile([D, QF], F32, tag="ps")
                for ci in range(NC):
                    nc.tensor.matmul(
                        pk, wk_sb[:, ci, :], xt_t[:, ci, :],
                        start=(ci == 0), stop=(ci == NC - 1),
                    )
                nc.vector.tensor_scalar(
                    out=kT_sb[:, sl], in0=pk, scalar1=bk_sb, scalar2=None,
                    op0=mybir.AluOpType.add,
                )
                pv = ps.tile([D, QF], F32, tag="ps")
                for ci in range(NC):
                    nc.tensor.matmul(
                        pv, wv_sb[:, ci, :], xt_t[:, ci, :],
                        start=(ci == 0), stop=(ci == NC - 1),
                    )
                nc.vector.tensor_scalar(
                    out=vT_sb[:, sl], in0=pv, scalar1=bv_sb, scalar2=None,
                    op0=mybir.AluOpType.add,
                )
                for j in range(KPC):
                    kt = sc * KPC + j
                    pt = ps.tile([P, D], BF16, tag="ps")
                    nc.tensor.transpose(
                        pt, vT_sb[:, kt * P:(kt + 1) * P], ident[0:D, 0:D]
                    )
                    nc.vector.tensor_copy(out=v_sb[:, kt, :], in_=pt)

            kv_chunk(0)
            for sc in range(NSC):
                for j in range(KPC):
                    kt = sc * KPC + j
                    sct = ps.tile([P, NQC, QF], F32, tag="ps")
                    for qc in range(NQC):
                        nc.tensor.matmul(
                            sct[:, qc, :],
                            kT_sb[:, kt * P:(kt + 1) * P],
                            qT_sb[:, qc * QF:(qc + 1) * QF],
                            start=True, stop=True,
                        )
                    exp_i = nc.scalar.activation(
                        out=e_sb[:, kt, :],
                        in_=sct[:, :, :],
                        func=mybir.ActivationFunctionType.Exp,
                        scale=float(SCALE),
                        accum_out=colsum[:, kt:kt + 1],
                    )
                    if kt == NK - 1:
                        last_exp = exp_i
                if sc + 1 < NSC:
                    kv_chunk(sc + 1)
                # issue the group's colsum AllReduce as soon as its exps done
                kt_done = (sc + 1) * KPC
                if kt_done in AR_BOUNDS:
                    g = AR_BOUNDS.index(kt_done)
                    kt_lo = 0 if g == 0 else AR_BOUNDS[g - 1]
                    gsl = slice(kt_lo, kt_done)
                    gn = kt_done - kt_lo
                    cs_in = dram.tile([P, gn], F32, tag=f"cs_in{g}")
                    cs_out = dram.tile([P, gn], F32, tag=f"cs_out{g}")
                    nc.sync.dma_start(out=cs_in, in_=colsum[:, gsl])
                    nc.gpsimd.collective_compute(
                        "AllReduce",
                        mybir.AluOpType.add,
                        replica_groups=[[0, 1], [2, 3], [4, 5], [6, 7]],
                        ins=[cs_in.opt()],
                        outs=[cs_out.opt()],
                    )
                    nc.sync.dma_start(out=cs_all[:, gsl], in_=cs_out)
                    nc.vector.reciprocal(out=recip[:, gsl], in_=cs_all[:, gsl])
                    for kt in range(kt_lo, kt_done):
                        nc.vector.tensor_scalar(
                            out=vs_sb[:, kt, :],
                            in0=v_sb[:, kt, :],
                            scalar1=recip[:, kt:kt + 1], scalar2=None,
                            op0=mybir.AluOpType.mult,
                        )

            # ---- attention matmuls, column-packed 2 k-tiles at a time -----
            po = ps.tile([P, NQC, QF], F32, tag="ps")
            npair = NK // 2
            for p_i in range(npair):
                ktA, ktB = 2 * p_i, 2 * p_i + 1
                for qc in range(NQC):
                    qsl = slice(qc * QF, (qc + 1) * QF)
                    mA = nc.tensor.matmul(
                        po[0:D, qc, :],
                        vs_sb[:, ktA, :],
                        e_sb[:, ktA, qsl],
                        start=(p_i == 0), stop=(p_i == npair - 1),
                        tile_position=(0, 0), skip_group_check=True,
                    )
                    mB = nc.tensor.matmul(
                        po[64:64 + D, qc, :],
                        vs_sb[:, ktB, :],
                        e_sb[:, ktB, qsl],
                        start=(p_i == 0), stop=(p_i == npair - 1),
                        tile_position=(0, 64), skip_group_check=True,
                    )
                    if p_i == 0 and qc == 0:
                        # keep attn's psum accumulator from grabbing a slot
                        # before the last scores tiles (slot starvation of
                        # the exp pipeline when the pair's AR returns early)
                        tile.add_dep_helper(
                            last_exp.ins, mA.ins, sync=False,
                            reason="attn after last exp",
                        )
                        tile.add_dep_helper(
                            last_exp.ins, mB.ins, sync=False,
                            reason="attn after last exp",
                        )
            nc.vector.tensor_copy(out=out_sb, in_=po[0:D, :, :])
            nc.vector.tensor_add(
                out=out_sb,
                in0=out_sb,
                in1=po[64:64 + D, :, :],
            )
            nc.sync.dma_start(out=out_d[:, :], in_=out_sb)

    nc.compile()
    return nc


_NC_CACHE = None


def _get_nc():
    global _NC_CACHE
    if _NC_CACHE is None:
        _NC_CACHE = _build()
    return _NC_CACHE


def kernel(x, Wq, bq, Wk, bk, Wv, bv):
    x = np.asarray(x, np.float32)
    bf = ml_dtypes.bfloat16
    w_bf = {
        "wq": np.ascontiguousarray(np.asarray(Wq, np.float32)).astype(bf),
        "wk": np.ascontiguousarray(np.asarray(Wk, np.float32)).astype(bf),
        "wv": np.ascontiguousarray(np.asarray(Wv, np.float32)).astype(bf),
    }
    b_f32 = {
        "bq": np.ascontiguousarray(np.asarray(bq, np.float32)).reshape(D, 1),
        "bk": np.ascontiguousarray(np.asarray(bk, np.float32)).reshape(D, 1),
        "bv": np.ascontiguousarray(np.asarray(bv, np.float32)).reshape(D, 1),
    }

    in_maps = []
    for core in range(N_CORES):
        b_idx, h = divmod(core, 2)
        xt = np.ascontiguousarray(x[b_idx].T).astype(bf)          # [768, 4096]
        xtq = np.ascontiguousarray(xt[:, h * SH:(h + 1) * SH])    # [768, 2048]
        in_maps.append({"xt": xt, "xtq": xtq, **w_bf, **b_f32})

    res = run_bass_kernel_spmd(
        _get_nc(), in_maps, core_ids=list(range(N_CORES)), trace=False
    )

    out = np.empty((B, S, D), np.float32)
    for core in range(N_CORES):
        b_idx, h = divmod(core, 2)
        out[b_idx, h * SH:(h + 1) * SH, :] = res.results[core]["out"].T
    return out


# revision 23
# speedup vs baseline: 1.3024x; 1.0109x over previous
"""Distributed Trainium2 Bass kernel for single-head attention with
softmax over the QUERY axis (faithful to the reference).

Reference math (per batch b):
    q = x @ Wq + bq          # [S, D]   S=4096, D=48
    k = x @ Wk + bk
    v = x @ Wv + bv
    s = (q @ k.T) / sqrt(D)  # [S_q, S_k]
    p = softmax(s, axis=QUERY)          # normalize each k-COLUMN over q
    out = p @ v              # [S_q, D]

Sharding: 8 cores = 4 batches x 2 query-halves. Core c handles batch
c//2, query rows [ (c%2)*2048, (c%2+1)*2048 ).

Layout: everything is computed TRANSPOSED on chip.
  - scores_T[k, q] tiles have k on partitions / q on the free axis, so
    the softmax denominator colsum[k] = sum_q exp(s[q,k]) is a free-axis
    reduction that the ScalarEngine emits for free via
    activation(Exp, accum_out=...).
  - The per-column normalization folds into V (V[k,:] /= colsum[k]).
  - colsum needs both query-halves: two 8 KB pairwise AllReduces,
    chunked over k so the first one hides under the exp phase.
  - Output is produced as out_T [48, 2048] and transposed on host.

Structure is ordered so the ScalarEngine (exp over 8.4M scores = the
roofline engine here) starts ~10us in and paces the whole middle of the
kernel: per 512-wide sequence chunk we emit [load xT chunk, K-proj,
V-proj, V-transposes, 4x (scores matmul + exp)], all overlapped by the
Tile scheduler. The attention matmuls are column-packed two-at-a-time
into the PE array (tile_position (0,0)/(0,64)) since their M=48 leaves
most of the array idle.

exp() runs without max-subtraction: scores*scale is N(0,~1/9), bounded
by ~|2.5| for these inputs, so exp stays well inside fp32 range
(softmax is shift-invariant, so the result matches the reference).
"""

import sys

for _p in ("/opt/trn_rl_repo",):
    if _p not in sys.path:
        sys.path.insert(0, _p)

import numpy as np
import ml_dtypes

import concourse.bass as bass
import concourse.tile as tile
from concourse import bacc, mybir
from concourse.bass_utils import run_bass_kernel_spmd
from concourse.masks import make_identity

N_CORES = 8
B = 4
S = 4096
DIM = 768
D = 48
SH = S // 2          # query rows per core
P = 128
NK = S // P          # 32 k-tiles
NC = DIM // P        # 6 contraction tiles for projections
QF = 512             # matmul moving free dim
NQC = SH // QF       # 4 q-chunks per core
NSC = S // QF        # 8 s-chunks for K/V projections
KPC = QF // P        # 4 k-tiles per s-chunk
SCALE = 1.0 / np.sqrt(np.float32(D))
# colsum AllReduce group boundaries (in k-tiles). Non-uniform: the first
# AR absorbs the ~34us collective warmup while exp still runs; later ARs
# pipeline at ~8.5us marginal, so only the last small group's AR+attn is
# exposed after the exp phase ends.
AR_BOUNDS = [16, 24, 32]

BF16 = mybir.dt.bfloat16
F32 = mybir.dt.float32


def _build():
    nc = bacc.Bacc(
        "TRN2",
        target_bir_lowering=False,
        debug=False,
        num_devices=N_CORES,
    )

    xt_d = nc.dram_tensor("xt", [DIM, S], BF16, kind="ExternalInput")
    xtq_d = nc.dram_tensor("xtq", [DIM, SH], BF16, kind="ExternalInput")
    wq_d = nc.dram_tensor("wq", [DIM, D], BF16, kind="ExternalInput")
    wkv_d = nc.dram_tensor("wkv", [DIM, 112], BF16, kind="ExternalInput")
    bq_d = nc.dram_tensor("bq", [D, 1], F32, kind="ExternalInput")
    bkv_d = nc.dram_tensor("bkv", [112, 1], F32, kind="ExternalInput")
    out_d = nc.dram_tensor("out", [D, SH], F32, kind="ExternalOutput")

    with tile.TileContext(nc) as tc:
        with (
            tc.tile_pool(name="consts", bufs=1) as consts,
            tc.tile_pool(name="big", bufs=1) as big,
            tc.tile_pool(name="xtp", bufs=3) as xtp,
            tc.tile_pool(name="ps", bufs=2, space="PSUM") as ps,
            tc.tile_pool(name="dram", bufs=1, space="DRAM") as dram,
        ):
            # ---- constants; Q-path DMAs first so exp starts early ---------
            wq_sb = consts.tile([P, NC, D], BF16, tag="wq")
            nc.sync.dma_start(out=wq_sb, in_=wq_d[:, :].rearrange("(i p) d -> p i d", p=P))
            bq_sb = consts.tile([D, 1], F32, tag="bq")
            nc.sync.dma_start(out=bq_sb, in_=bq_d[:, :])
            wkv_sb = consts.tile([P, NC, 112], BF16, tag="wkv")
            nc.sync.dma_start(out=wkv_sb, in_=wkv_d[:, :].rearrange("(i p) d -> p i d", p=P))
            bkv_sb = consts.tile([112, 1], F32, tag="bkv")
            nc.sync.dma_start(out=bkv_sb, in_=bkv_d[:, :])
            ident = consts.tile([P, P], BF16, tag="ident")
            make_identity(nc, ident)

            # ---- persistent SBUF tensors ----------------------------------
            kT_sb = big.tile([D, S], BF16, tag="kT")      # K^T  [48, 4096]
            vT_sb = big.tile([112, S], BF16, tag="vT")    # V^T at partitions 64-111
            qT_sb = big.tile([D, SH], BF16, tag="qT")     # Q^T  [48, 2048]
            v_sb = big.tile([P, NK, D], BF16, tag="v")    # V    [k, d] tiles
            vs_sb = big.tile([P, NK, D], BF16, tag="vs")  # V / colsum
            e_sb = big.tile([P, NK, SH], BF16, tag="e")   # E_T  [k, q] tiles
            colsum = big.tile([P, NK], F32, tag="colsum")
            recip = big.tile([P, NK], F32, tag="recip")
            cs_all = big.tile([P, NK], F32, tag="cs_all")
            out_sb = big.tile([D, SH], F32, tag="out")

            # ---- Q^T projection (own query half) --------------------------
            for qc in range(NQC):
                xq_t = xtp.tile([P, NC, QF], BF16, tag="xt")
                nc.sync.dma_start(
                    out=xq_t,
                    in_=xtq_d[:, qc * QF:(qc + 1) * QF].rearrange(
                        "(i p) f -> p i f", p=P
                    ),
                )
                pq = ps.tile([D, QF], F32, tag="ps")
                for ci in range(NC):
                    nc.tensor.matmul(
                        pq, wq_sb[:, ci, :], xq_t[:, ci, :],
                        start=(ci == 0), stop=(ci == NC - 1),
                    )
                nc.vector.tensor_scalar(
                    out=qT_sb[:, qc * QF:(qc + 1) * QF],
                    in0=pq, scalar1=bq_sb, scalar2=None,
                    op0=mybir.AluOpType.add,
                )

            # ---- software-pipelined: proj(sc+1) overlaps scores/exp(sc) --
            def kv_chunk(sc):
                sl = slice(sc * QF, (sc + 1) * QF)
                xt_t = xtp.tile([P, NC, QF], BF16, tag="xt")
                nc.sync.dma_start(
                    out=xt_t,
                    in_=xt_d[:, sl].rearrange("(i p) f -> p i f", p=P),
                )
                # fused K|V projection (V padded to array cols 64-111 so
                # both epilogue reads land on 32-aligned partition bases)
                pkv = ps.tile([112, QF], F32, tag="ps")
                for ci in range(NC):
                    nc.tensor.matmul(
                        pkv, wkv_sb[:, ci, :], xt_t[:, ci, :],
                        start=(ci == 0), stop=(ci == NC - 1),
                    )
                nc.vector.tensor_scalar(
                    out=kT_sb[:, sl], in0=pkv[0:D, :], scalar1=bkv_sb[0:D, :],
                    scalar2=None, op0=mybir.AluOpType.add,
                )
                nc.vector.tensor_scalar(
                    out=vT_sb[64:64 + D, sl], in0=pkv[64:64 + D, :],
                    scalar1=bkv_sb[64:64 + D, :], scalar2=None,
                    op0=mybir.AluOpType.add,
                )
                for j in range(KPC):
                    kt = sc * KPC + j
                    pt = ps.tile([P, D], BF16, tag="ps")
                    nc.tensor.transpose(
                        pt, vT_sb[64:64 + D, kt * P:(kt + 1) * P],
                        ident[64:64 + D, 64:64 + D],
                        tile_position=(64, 0),
                    )
                    nc.vector.tensor_copy(out=v_sb[:, kt, :], in_=pt)

            kv_chunk(0)
            for sc in range(NSC):
                for j in range(KPC):
                    kt = sc * KPC + j
                    sct = ps.tile([P, NQC, QF], F32, tag="ps")
                    for qc in range(NQC):
                        nc.tensor.matmul(
                            sct[:, qc, :],
                            kT_sb[:, kt * P:(kt + 1) * P],
                            qT_sb[:, qc * QF:(qc + 1) * QF],
                            start=True, stop=True,
                        )
                    exp_i = nc.scalar.activation(
                        out=e_sb[:, kt, :],
                        in_=sct[:, :, :],
                        func=mybir.ActivationFunctionType.Exp,
                        scale=float(SCALE),
                        accum_out=colsum[:, kt:kt + 1],
                    )
                    if kt == NK - 1:
                        last_exp = exp_i
                if sc + 1 < NSC:
                    kv_chunk(sc + 1)
                # issue the group's colsum AllReduce as soon as its exps done
                kt_done = (sc + 1) * KPC
                if kt_done in AR_BOUNDS:
                    g = AR_BOUNDS.index(kt_done)
                    kt_lo = 0 if g == 0 else AR_BOUNDS[g - 1]
                    gsl = slice(kt_lo, kt_done)
                    gn = kt_done - kt_lo
                    cs_in = dram.tile([P, gn], F32, tag=f"cs_in{g}")
                    cs_out = dram.tile([P, gn], F32, tag=f"cs_out{g}")
                    nc.sync.dma_start(out=cs_in, in_=colsum[:, gsl])
                    nc.gpsimd.collective_compute(
                        "AllReduce",
                        mybir.AluOpType.add,
                        replica_groups=[[0, 1], [2, 3], [4, 5], [6, 7]],
                        ins=[cs_in.opt()],
                        outs=[cs_out.opt()],
                    )
                    nc.sync.dma_start(out=cs_all[:, gsl], in_=cs_out)
                    nc.vector.reciprocal(out=recip[:, gsl], in_=cs_all[:, gsl])
                    for kt in range(kt_lo, kt_done):
                        nc.vector.tensor_scalar(
                            out=vs_sb[:, kt, :],
                            in0=v_sb[:, kt, :],
                            scalar1=recip[:, kt:kt + 1], scalar2=None,
                            op0=mybir.AluOpType.mult,
                        )

            # ---- attention matmuls, column-packed 2 k-tiles at a time -----
            po = ps.tile([P, NQC, QF], F32, tag="ps")
            npair = NK // 2
            for p_i in range(npair):
                ktA, ktB = 2 * p_i, 2 * p_i + 1
                for qc in range(NQC):
                    qsl = slice(qc * QF, (qc + 1) * QF)
                    mA = nc.tensor.matmul(
                        po[0:D, qc, :],
                        vs_sb[:, ktA, :],
                        e_sb[:, ktA, qsl],
                        start=(p_i == 0), stop=(p_i == npair - 1),
                        tile_position=(0, 0), skip_group_check=True,
                    )
                    mB = nc.tensor.matmul(
                        po[64:64 + D, qc, :],
                        vs_sb[:, ktB, :],
                        e_sb[:, ktB, qsl],
                        start=(p_i == 0), stop=(p_i == npair - 1),
                        tile_position=(0, 64), skip_group_check=True,
                    )
                    if p_i == 0 and qc == 0:
                        # keep attn's psum accumulator from grabbing a slot
                        # before the last scores tiles (slot starvation of
                        # the exp pipeline when the pair's AR returns early)
                        tile.add_dep_helper(
                            last_exp.ins, mA.ins, sync=False,
                            reason="attn after last exp",
                        )
                        tile.add_dep_helper(
                            last_exp.ins, mB.ins, sync=False,
                            reason="attn after last exp",
                        )
            nc.vector.tensor_copy(out=out_sb, in_=po[0:D, :, :])
            nc.vector.tensor_add(
                out=out_sb,
                in0=out_sb,
                in1=po[64:64 + D, :, :],
            )
            nc.sync.dma_start(out=out_d[:, :], in_=out_sb)

    nc.compile()
    return nc


_NC_CACHE = None


def _get_nc():
    global _NC_CACHE
    if _NC_CACHE is None:
        _NC_CACHE = _build()
    return _NC_CACHE


def kernel(x, Wq, bq, Wk, bk, Wv, bv):
    x = np.asarray(x, np.float32)
    bf = ml_dtypes.bfloat16
    wkv = np.zeros((DIM, 112), np.float32)
    wkv[:, 0:D] = np.asarray(Wk, np.float32)
    wkv[:, 64:64 + D] = np.asarray(Wv, np.float32)
    bkv = np.zeros((112,), np.float32)
    bkv[0:D] = np.asarray(bk, np.float32).ravel()
    bkv[64:64 + D] = np.asarray(bv, np.float32).ravel()
    w_bf = {
        "wq": np.ascontiguousarray(np.asarray(Wq, np.float32)).astype(bf),
        "wkv": np.ascontiguousarray(wkv).astype(bf),
    }
    b_f32 = {
        "bq": np.ascontiguousarray(np.asarray(bq, np.float32)).reshape(D, 1),
        "bkv": np.ascontiguousarray(bkv).reshape(112, 1),
    }

    in_maps = []
    for core in range(N_CORES):
        b_idx, h = divmod(core, 2)
        xt = np.ascontiguousarray(x[b_idx].T).astype(bf)          # [768, 4096]
        xtq = np.ascontiguousarray(xt[:, h * SH:(h + 1) * SH])    # [768, 2048]
        in_maps.append({"xt": xt, "xtq": xtq, **w_bf, **b_f32})

    res = run_bass_kernel_spmd(
        _get_nc(), in_maps, core_ids=list(range(N_CORES)), trace=False
    )

    out = np.empty((B, S, D), np.float32)
    for core in range(N_CORES):
        b_idx, h = divmod(core, 2)
        out[b_idx, h * SH:(h + 1) * SH, :] = res.results[core]["out"].T
    return out


# revision 26
# speedup vs baseline: 1.4023x; 1.0767x over previous
"""Distributed Trainium2 Bass kernel for single-head attention with
softmax over the QUERY axis (faithful to the reference).

Reference math (per batch b):
    q = x @ Wq + bq          # [S, D]   S=4096, D=48
    k = x @ Wk + bk
    v = x @ Wv + bv
    s = (q @ k.T) / sqrt(D)  # [S_q, S_k]
    p = softmax(s, axis=QUERY)          # normalize each k-COLUMN over q
    out = p @ v              # [S_q, D]

Sharding: 8 cores = 4 batches x 2 query-halves. Core c handles batch
c//2, query rows [ (c%2)*2048, (c%2+1)*2048 ).

Layout: everything is computed TRANSPOSED on chip.
  - scores_T[k, q] tiles have k on partitions / q on the free axis, so
    the softmax denominator colsum[k] = sum_q exp(s[q,k]) is a free-axis
    reduction that the ScalarEngine emits for free via
    activation(Exp, accum_out=...).
  - The per-column normalization folds into V (V[k,:] /= colsum[k]).
  - colsum needs both query-halves: two 8 KB pairwise AllReduces,
    chunked over k so the first one hides under the exp phase.
  - Output is produced as out_T [48, 2048] and transposed on host.

Structure is ordered so the ScalarEngine (exp over 8.4M scores = the
roofline engine here) starts ~10us in and paces the whole middle of the
kernel: per 512-wide sequence chunk we emit [load xT chunk, K-proj,
V-proj, V-transposes, 4x (scores matmul + exp)], all overlapped by the
Tile scheduler. The attention matmuls are column-packed two-at-a-time
into the PE array (tile_position (0,0)/(0,64)) since their M=48 leaves
most of the array idle.

exp() runs without max-subtraction: scores*scale is N(0,~1/9), bounded
by ~|2.5| for these inputs, so exp stays well inside fp32 range
(softmax is shift-invariant, so the result matches the reference).
"""

import sys

for _p in ("/opt/trn_rl_repo",):
    if _p not in sys.path:
        sys.path.insert(0, _p)

import numpy as np
import ml_dtypes

import concourse.bass as bass
import concourse.tile as tile
from concourse import bacc, mybir
from concourse.bass_utils import run_bass_kernel_spmd
from concourse.masks import make_identity

N_CORES = 8
B = 4
S = 4096
DIM = 768
D = 48
SH = S // 2          # query rows per core
P = 128
NK = S // P          # 32 k-tiles
NC = DIM // P        # 6 contraction tiles for projections
QF = 512             # matmul moving free dim
NQC = SH // QF       # 4 q-chunks per core
NSC = S // QF        # 8 s-chunks for K/V projections
KPC = QF // P        # 4 k-tiles per s-chunk
SCALE = 1.0 / np.sqrt(np.float32(D))
# colsum AllReduce group boundaries (in k-tiles). Non-uniform: the first
# AR absorbs the ~34us collective warmup while exp still runs; later ARs
# pipeline at ~8.5us marginal, so only the last small group's AR+attn is
# exposed after the exp phase ends.
AR_BOUNDS = [16, 24, 32]

BF16 = mybir.dt.bfloat16
F32 = mybir.dt.float32


def _build():
    nc = bacc.Bacc(
        "TRN2",
        target_bir_lowering=False,
        debug=False,
        num_devices=N_CORES,
    )

    xt_d = nc.dram_tensor("xt", [DIM, S], BF16, kind="ExternalInput")
    xtq_d = nc.dram_tensor("xtq", [DIM, SH], BF16, kind="ExternalInput")
    wq_d = nc.dram_tensor("wq", [DIM, D], BF16, kind="ExternalInput")
    wkv_d = nc.dram_tensor("wkv", [DIM, 112], BF16, kind="ExternalInput")
    bq_d = nc.dram_tensor("bq", [D, 1], F32, kind="ExternalInput")
    bkv_d = nc.dram_tensor("bkv", [112, 1], F32, kind="ExternalInput")
    out_d = nc.dram_tensor("out", [D, SH], F32, kind="ExternalOutput")

    with tile.TileContext(nc) as tc:
        with (
            tc.tile_pool(name="consts", bufs=1) as consts,
            tc.tile_pool(name="big", bufs=1) as big,
            tc.tile_pool(name="xtp", bufs=3) as xtp,
            tc.tile_pool(name="ps", bufs=3, space="PSUM") as ps,
            tc.tile_pool(name="pj", bufs=2, space="PSUM") as pj,
            tc.tile_pool(name="dram", bufs=1, space="DRAM") as dram,
        ):
            # ---- constants; Q-path DMAs first so exp starts early ---------
            wq_sb = consts.tile([P, NC, D], BF16, tag="wq")
            nc.sync.dma_start(out=wq_sb, in_=wq_d[:, :].rearrange("(i p) d -> p i d", p=P))
            bq_sb = consts.tile([D, 1], F32, tag="bq")
            nc.sync.dma_start(out=bq_sb, in_=bq_d[:, :])
            wkv_sb = consts.tile([P, NC, 112], BF16, tag="wkv")
            nc.sync.dma_start(out=wkv_sb, in_=wkv_d[:, :].rearrange("(i p) d -> p i d", p=P))
            bkv_sb = consts.tile([112, 1], F32, tag="bkv")
            nc.sync.dma_start(out=bkv_sb, in_=bkv_d[:, :])
            ident = consts.tile([P, P], BF16, tag="ident")
            make_identity(nc, ident)

            # ---- persistent SBUF tensors ----------------------------------
            kT_sb = big.tile([D, S], BF16, tag="kT")      # K^T  [48, 4096]
            vT_sb = big.tile([112, S], BF16, tag="vT")    # V^T at partitions 64-111
            qT_sb = big.tile([D, SH], BF16, tag="qT")     # Q^T  [48, 2048]
            v_sb = big.tile([P, NK, D], BF16, tag="v")    # V    [k, d] tiles
            vs_sb = big.tile([P, NK, D], BF16, tag="vs")  # V / colsum
            e_sb = big.tile([P, NK, SH], BF16, tag="e")   # E_T  [k, q] tiles
            colsum = big.tile([P, NK], F32, tag="colsum")
            colsumh = big.tile([P, NK, 2], F32, tag="colsumh")
            recip = big.tile([P, NK], F32, tag="recip")
            cs_all = big.tile([P, NK], F32, tag="cs_all")
            out_sb = big.tile([D, SH], F32, tag="out")

            # ---- Q^T projection (own query half) --------------------------
            for qc in range(NQC):
                xq_t = xtp.tile([P, NC, QF], BF16, tag="xt")
                nc.sync.dma_start(
                    out=xq_t,
                    in_=xtq_d[:, qc * QF:(qc + 1) * QF].rearrange(
                        "(i p) f -> p i f", p=P
                    ),
                )
                pq = pj.tile([D, QF], F32, tag="pj")
                for ci in range(NC):
                    nc.tensor.matmul(
                        pq, wq_sb[:, ci, :], xq_t[:, ci, :],
                        start=(ci == 0), stop=(ci == NC - 1),
                    )
                nc.vector.tensor_scalar(
                    out=qT_sb[:, qc * QF:(qc + 1) * QF],
                    in0=pq, scalar1=bq_sb, scalar2=None,
                    op0=mybir.AluOpType.add,
                )

            # ---- software-pipelined: proj(sc+1) overlaps scores/exp(sc) --
            def kv_chunk(sc):
                sl = slice(sc * QF, (sc + 1) * QF)
                xt_t = xtp.tile([P, NC, QF], BF16, tag="xt")
                nc.sync.dma_start(
                    out=xt_t,
                    in_=xt_d[:, sl].rearrange("(i p) f -> p i f", p=P),
                )
                # fused K|V projection (V padded to array cols 64-111 so
                # both epilogue reads land on 32-aligned partition bases)
                pkv = pj.tile([112, QF], F32, tag="pj")
                for ci in range(NC):
                    nc.tensor.matmul(
                        pkv, wkv_sb[:, ci, :], xt_t[:, ci, :],
                        start=(ci == 0), stop=(ci == NC - 1),
                    )
                nc.vector.tensor_scalar(
                    out=kT_sb[:, sl], in0=pkv[0:D, :], scalar1=bkv_sb[0:D, :],
                    scalar2=None, op0=mybir.AluOpType.add,
                )
                nc.vector.tensor_scalar(
                    out=vT_sb[64:64 + D, sl], in0=pkv[64:64 + D, :],
                    scalar1=bkv_sb[64:64 + D, :], scalar2=None,
                    op0=mybir.AluOpType.add,
                )
                for j in range(KPC):
                    kt = sc * KPC + j
                    pt = pj.tile([P, D], BF16, tag="pj")
                    nc.tensor.transpose(
                        pt, vT_sb[64:64 + D, kt * P:(kt + 1) * P],
                        ident[64:64 + D, 64:64 + D],
                        tile_position=(64, 0),
                    )
                    nc.vector.tensor_copy(out=v_sb[:, kt, :], in_=pt)

            kv_chunk(0)
            for sc in range(NSC):
                for j in range(KPC):
                    kt = sc * KPC + j
                    for h in range(2):
                        sct = ps.tile([P, 2, QF], F32, tag="ps")
                        for qh in range(2):
                            qc = 2 * h + qh
                            nc.tensor.matmul(
                                sct[:, qh, :],
                                kT_sb[:, kt * P:(kt + 1) * P],
                                qT_sb[:, qc * QF:(qc + 1) * QF],
                                start=True, stop=True,
                            )
                        exp_i = nc.scalar.activation(
                            out=e_sb[:, kt, h * 2 * QF:(h + 1) * 2 * QF],
                            in_=sct[:, :, :],
                            func=mybir.ActivationFunctionType.Exp,
                            scale=float(SCALE),
                            accum_out=colsumh[:, kt, h:h + 1],
                        )
                        if kt == NK - 1 and h == 1:
                            last_exp = exp_i
                if sc + 1 < NSC:
                    kv_chunk(sc + 1)
                # issue the group's colsum AllReduce as soon as its exps done
                kt_done = (sc + 1) * KPC
                if kt_done in AR_BOUNDS:
                    g = AR_BOUNDS.index(kt_done)
                    kt_lo = 0 if g == 0 else AR_BOUNDS[g - 1]
                    gsl = slice(kt_lo, kt_done)
                    gn = kt_done - kt_lo
                    nc.vector.tensor_add(
                        out=colsum[:, gsl],
                        in0=colsumh[:, gsl, 0],
                        in1=colsumh[:, gsl, 1],
                    )
                    cs_in = dram.tile([P, gn], F32, tag=f"cs_in{g}")
                    cs_out = dram.tile([P, gn], F32, tag=f"cs_out{g}")
                    nc.sync.dma_start(out=cs_in, in_=colsum[:, gsl])
                    nc.gpsimd.collective_compute(
                        "AllReduce",
                        mybir.AluOpType.add,
                        replica_groups=[[0, 1], [2, 3], [4, 5], [6, 7]],
                        ins=[cs_in.opt()],
                        outs=[cs_out.opt()],
                    )
                    nc.sync.dma_start(out=cs_all[:, gsl], in_=cs_out)
                    nc.vector.reciprocal(out=recip[:, gsl], in_=cs_all[:, gsl])
                    for kt in range(kt_lo, kt_done):
                        nc.vector.tensor_scalar(
                            out=vs_sb[:, kt, :],
                            in0=v_sb[:, kt, :],
                            scalar1=recip[:, kt:kt + 1], scalar2=None,
                            op0=mybir.AluOpType.mult,
                        )

            # ---- attention matmuls, column-packed 2 k-tiles at a time -----
            po_a = ps.tile([P, 2, QF], F32, tag="ps")
            po_b = ps.tile([P, 2, QF], F32, tag="ps")
            if True:
                npair = NK // 2
                for p_i in range(npair):
                    ktA, ktB = 2 * p_i, 2 * p_i + 1
                    for qc in range(NQC):
                        qsl = slice(qc * QF, (qc + 1) * QF)
                        pot = po_a if qc < 2 else po_b
                        mA = nc.tensor.matmul(
                            pot[0:D, qc % 2, :],
                            vs_sb[:, ktA, :],
                            e_sb[:, ktA, qsl],
                            start=(p_i == 0), stop=(p_i == npair - 1),
                            tile_position=(0, 0), skip_group_check=True,
                        )
                        mB = nc.tensor.matmul(
                            pot[64:64 + D, qc % 2, :],
                            vs_sb[:, ktB, :],
                            e_sb[:, ktB, qsl],
                            start=(p_i == 0), stop=(p_i == npair - 1),
                            tile_position=(0, 64), skip_group_check=True,
                        )
                        if p_i == 0 and qc == 0:
                            tile.add_dep_helper(
                                last_exp.ins, mA.ins, sync=False,
                                reason="attn after last exp",
                            )
                            tile.add_dep_helper(
                                last_exp.ins, mB.ins, sync=False,
                                reason="attn after last exp",
                            )
                out3 = out_sb.rearrange("d (c f) -> d c f", f=QF)
                nc.vector.tensor_copy(out=out3[:, 0:2, :], in_=po_a[0:D, :, :])
                nc.vector.tensor_copy(out=out3[:, 2:4, :], in_=po_b[0:D, :, :])
                nc.vector.tensor_add(
                    out=out3[:, 0:2, :], in0=out3[:, 0:2, :],
                    in1=po_a[64:64 + D, :, :],
                )
                nc.vector.tensor_add(
                    out=out3[:, 2:4, :], in0=out3[:, 2:4, :],
                    in1=po_b[64:64 + D, :, :],
                )
                nc.sync.dma_start(out=out_d[:, :], in_=out_sb)

    nc.compile()
    return nc


_NC_CACHE = None


def _get_nc():
    global _NC_CACHE
    if _NC_CACHE is None:
        _NC_CACHE = _build()
    return _NC_CACHE


def kernel(x, Wq, bq, Wk, bk, Wv, bv):
    x = np.asarray(x, np.float32)
    bf = ml_dtypes.bfloat16
    wkv = np.zeros((DIM, 112), np.float32)
    wkv[:, 0:D] = np.asarray(Wk, np.float32)
    wkv[:, 64:64 + D] = np.asarray(Wv, np.float32)
    bkv = np.zeros((112,), np.float32)
    bkv[0:D] = np.asarray(bk, np.float32).ravel()
    bkv[64:64 + D] = np.asarray(bv, np.float32).ravel()
    w_bf = {
        "wq": np.ascontiguousarray(np.asarray(Wq, np.float32)).astype(bf),
        "wkv": np.ascontiguousarray(wkv).astype(bf),
    }
    b_f32 = {
        "bq": np.ascontiguousarray(np.asarray(bq, np.float32)).reshape(D, 1),
        "bkv": np.ascontiguousarray(bkv).reshape(112, 1),
    }

    in_maps = []
    for core in range(N_CORES):
        b_idx, h = divmod(core, 2)
        xt = np.ascontiguousarray(x[b_idx].T).astype(bf)          # [768, 4096]
        xtq = np.ascontiguousarray(xt[:, h * SH:(h + 1) * SH])    # [768, 2048]
        in_maps.append({"xt": xt, "xtq": xtq, **w_bf, **b_f32})

    res = run_bass_kernel_spmd(
        _get_nc(), in_maps, core_ids=list(range(N_CORES)), trace=False
    )

    out = np.empty((B, S, D), np.float32)
    for core in range(N_CORES):
        b_idx, h = divmod(core, 2)
        out[b_idx, h * SH:(h + 1) * SH, :] = res.results[core]["out"].T
    return out


# revision 27
# speedup vs baseline: 1.4647x; 1.0445x over previous
"""Distributed Trainium2 Bass kernel for single-head attention with
softmax over the QUERY axis (faithful to the reference).

Reference math (per batch b):
    q = x @ Wq + bq          # [S, D]   S=4096, D=48
    k = x @ Wk + bk
    v = x @ Wv + bv
    s = (q @ k.T) / sqrt(D)  # [S_q, S_k]
    p = softmax(s, axis=QUERY)          # normalize each k-COLUMN over q
    out = p @ v              # [S_q, D]

Sharding: 8 cores = 4 batches x 2 query-halves. Core c handles batch
c//2, query rows [ (c%2)*2048, (c%2+1)*2048 ).

Layout: everything is computed TRANSPOSED on chip.
  - scores_T[k, q] tiles have k on partitions / q on the free axis, so
    the softmax denominator colsum[k] = sum_q exp(s[q,k]) is a free-axis
    reduction that the ScalarEngine emits for free via
    activation(Exp, accum_out=...).
  - The per-column normalization folds into V (V[k,:] /= colsum[k]).
  - colsum needs both query-halves: two 8 KB pairwise AllReduces,
    chunked over k so the first one hides under the exp phase.
  - Output is produced as out_T [48, 2048] and transposed on host.

Structure is ordered so the ScalarEngine (exp over 8.4M scores = the
roofline engine here) starts ~10us in and paces the whole middle of the
kernel: per 512-wide sequence chunk we emit [load xT chunk, K-proj,
V-proj, V-transposes, 4x (scores matmul + exp)], all overlapped by the
Tile scheduler. The attention matmuls are column-packed two-at-a-time
into the PE array (tile_position (0,0)/(0,64)) since their M=48 leaves
most of the array idle.

exp() runs without max-subtraction: scores*scale is N(0,~1/9), bounded
by ~|2.5| for these inputs, so exp stays well inside fp32 range
(softmax is shift-invariant, so the result matches the reference).
"""

import sys

for _p in ("/opt/trn_rl_repo",):
    if _p not in sys.path:
        sys.path.insert(0, _p)

import numpy as np
import ml_dtypes

import concourse.bass as bass
import concourse.tile as tile
from concourse import bacc, mybir
from concourse.bass_utils import run_bass_kernel_spmd
from concourse.masks import make_identity

N_CORES = 8
B = 4
S = 4096
DIM = 768
D = 48
SH = S // 2          # query rows per core
P = 128
NK = S // P          # 32 k-tiles
NC = DIM // P        # 6 contraction tiles for projections
QF = 512             # matmul moving free dim
NQC = SH // QF       # 4 q-chunks per core
NSC = S // QF        # 8 s-chunks for K/V projections
KPC = QF // P        # 4 k-tiles per s-chunk
SCALE = 1.0 / np.sqrt(np.float32(D))
# colsum AllReduce group boundaries (in k-tiles). Non-uniform: the first
# AR absorbs the ~34us collective warmup while exp still runs; later ARs
# pipeline at ~8.5us marginal, so only the last small group's AR+attn is
# exposed after the exp phase ends.
AR_BOUNDS = [16, 24, 28, 32]

BF16 = mybir.dt.bfloat16
F32 = mybir.dt.float32


def _build():
    nc = bacc.Bacc(
        "TRN2",
        target_bir_lowering=False,
        debug=False,
        num_devices=N_CORES,
    )

    xt_d = nc.dram_tensor("xt", [DIM, S], BF16, kind="ExternalInput")
    xtq_d = nc.dram_tensor("xtq", [DIM, SH], BF16, kind="ExternalInput")
    wq_d = nc.dram_tensor("wq", [DIM, D], BF16, kind="ExternalInput")
    wkv_d = nc.dram_tensor("wkv", [DIM, 112], BF16, kind="ExternalInput")
    bq_d = nc.dram_tensor("bq", [D, 1], F32, kind="ExternalInput")
    bkv_d = nc.dram_tensor("bkv", [112, 1], F32, kind="ExternalInput")
    out_d = nc.dram_tensor("out", [D, SH], F32, kind="ExternalOutput")

    with tile.TileContext(nc) as tc:
        with (
            tc.tile_pool(name="consts", bufs=1) as consts,
            tc.tile_pool(name="big", bufs=1) as big,
            tc.tile_pool(name="xtp", bufs=3) as xtp,
            tc.tile_pool(name="ps", bufs=3, space="PSUM") as ps,
            tc.tile_pool(name="pj", bufs=2, space="PSUM") as pj,
            tc.tile_pool(name="dram", bufs=1, space="DRAM") as dram,
        ):
            # ---- constants; Q-path DMAs first so exp starts early ---------
            wq_sb = consts.tile([P, NC, D], BF16, tag="wq")
            nc.sync.dma_start(out=wq_sb, in_=wq_d[:, :].rearrange("(i p) d -> p i d", p=P))
            bq_sb = consts.tile([D, 1], F32, tag="bq")
            nc.sync.dma_start(out=bq_sb, in_=bq_d[:, :])
            wkv_sb = consts.tile([P, NC, 112], BF16, tag="wkv")
            nc.sync.dma_start(out=wkv_sb, in_=wkv_d[:, :].rearrange("(i p) d -> p i d", p=P))
            bkv_sb = consts.tile([112, 1], F32, tag="bkv")
            nc.sync.dma_start(out=bkv_sb, in_=bkv_d[:, :])
            ident = consts.tile([P, P], BF16, tag="ident")
            make_identity(nc, ident)

            # ---- persistent SBUF tensors ----------------------------------
            kT_sb = big.tile([D, S], BF16, tag="kT")      # K^T  [48, 4096]
            vT_sb = big.tile([112, S], BF16, tag="vT")    # V^T at partitions 64-111
            qT_sb = big.tile([D, SH], BF16, tag="qT")     # Q^T  [48, 2048]
            v_sb = big.tile([P, NK, D], BF16, tag="v")    # V    [k, d] tiles
            vs_sb = big.tile([P, NK, D], BF16, tag="vs")  # V / colsum
            e_sb = big.tile([P, NK, SH], BF16, tag="e")   # E_T  [k, q] tiles
            colsum = big.tile([P, NK], F32, tag="colsum")
            colsumh = big.tile([P, NK, 2], F32, tag="colsumh")
            recip = big.tile([P, NK], F32, tag="recip")
            cs_all = big.tile([P, NK], F32, tag="cs_all")
            out_sb = big.tile([D, SH], F32, tag="out")

            # ---- Q^T projection (own query half) --------------------------
            def q_chunk(qc):
                xq_t = xtp.tile([P, NC, QF], BF16, tag="xt")
                nc.sync.dma_start(
                    out=xq_t,
                    in_=xtq_d[:, qc * QF:(qc + 1) * QF].rearrange(
                        "(i p) f -> p i f", p=P
                    ),
                )
                pq = pj.tile([D, QF], F32, tag="pj")
                for ci in range(NC):
                    nc.tensor.matmul(
                        pq, wq_sb[:, ci, :], xq_t[:, ci, :],
                        start=(ci == 0), stop=(ci == NC - 1),
                    )
                nc.vector.tensor_scalar(
                    out=qT_sb[:, qc * QF:(qc + 1) * QF],
                    in0=pq, scalar1=bq_sb, scalar2=None,
                    op0=mybir.AluOpType.add,
                )

            q_chunk(0)
            q_chunk(1)

            # ---- software-pipelined: proj(sc+1) overlaps scores/exp(sc) --
            def kv_chunk(sc):
                sl = slice(sc * QF, (sc + 1) * QF)
                xt_t = xtp.tile([P, NC, QF], BF16, tag="xt")
                nc.sync.dma_start(
                    out=xt_t,
                    in_=xt_d[:, sl].rearrange("(i p) f -> p i f", p=P),
                )
                # fused K|V projection (V padded to array cols 64-111 so
                # both epilogue reads land on 32-aligned partition bases)
                pkv = pj.tile([112, QF], F32, tag="pj")
                for ci in range(NC):
                    nc.tensor.matmul(
                        pkv, wkv_sb[:, ci, :], xt_t[:, ci, :],
                        start=(ci == 0), stop=(ci == NC - 1),
                    )
                nc.vector.tensor_scalar(
                    out=kT_sb[:, sl], in0=pkv[0:D, :], scalar1=bkv_sb[0:D, :],
                    scalar2=None, op0=mybir.AluOpType.add,
                )
                nc.vector.tensor_scalar(
                    out=vT_sb[64:64 + D, sl], in0=pkv[64:64 + D, :],
                    scalar1=bkv_sb[64:64 + D, :], scalar2=None,
                    op0=mybir.AluOpType.add,
                )
                for j in range(KPC):
                    kt = sc * KPC + j
                    pt = pj.tile([P, D], BF16, tag="pj")
                    nc.tensor.transpose(
                        pt, vT_sb[64:64 + D, kt * P:(kt + 1) * P],
                        ident[64:64 + D, 64:64 + D],
                        tile_position=(64, 0),
                    )
                    nc.vector.tensor_copy(out=v_sb[:, kt, :], in_=pt)

            kv_chunk(0)
            q_chunk(2)
            q_chunk(3)
            for sc in range(NSC):
                for j in range(KPC):
                    kt = sc * KPC + j
                    for h in range(2):
                        sct = ps.tile([P, 2, QF], F32, tag="ps")
                        for qh in range(2):
                            qc = 2 * h + qh
                            nc.tensor.matmul(
                                sct[:, qh, :],
                                kT_sb[:, kt * P:(kt + 1) * P],
                                qT_sb[:, qc * QF:(qc + 1) * QF],
                                start=True, stop=True,
                            )
                        exp_i = nc.scalar.activation(
                            out=e_sb[:, kt, h * 2 * QF:(h + 1) * 2 * QF],
                            in_=sct[:, :, :],
                            func=mybir.ActivationFunctionType.Exp,
                            scale=float(SCALE),
                            accum_out=colsumh[:, kt, h:h + 1],
                        )
                        if kt == NK - 1 and h == 1:
                            last_exp = exp_i
                if sc + 1 < NSC:
                    kv_chunk(sc + 1)
                # issue the group's colsum AllReduce as soon as its exps done
                kt_done = (sc + 1) * KPC
                if kt_done in AR_BOUNDS:
                    g = AR_BOUNDS.index(kt_done)
                    kt_lo = 0 if g == 0 else AR_BOUNDS[g - 1]
                    gsl = slice(kt_lo, kt_done)
                    gn = kt_done - kt_lo
                    nc.vector.tensor_add(
                        out=colsum[:, gsl],
                        in0=colsumh[:, gsl, 0],
                        in1=colsumh[:, gsl, 1],
                    )
                    cs_in = dram.tile([P, gn], F32, tag=f"cs_in{g}")
                    cs_out = dram.tile([P, gn], F32, tag=f"cs_out{g}")
                    nc.sync.dma_start(out=cs_in, in_=colsum[:, gsl])
                    nc.gpsimd.collective_compute(
                        "AllReduce",
                        mybir.AluOpType.add,
                        replica_groups=[[0, 1], [2, 3], [4, 5], [6, 7]],
                        ins=[cs_in.opt()],
                        outs=[cs_out.opt()],
                    )
                    nc.sync.dma_start(out=cs_all[:, gsl], in_=cs_out)
                    nc.vector.reciprocal(out=recip[:, gsl], in_=cs_all[:, gsl])
                    for kt in range(kt_lo, kt_done):
                        nc.vector.tensor_scalar(
                            out=vs_sb[:, kt, :],
                            in0=v_sb[:, kt, :],
                            scalar1=recip[:, kt:kt + 1], scalar2=None,
                            op0=mybir.AluOpType.mult,
                        )

            # ---- attention matmuls, column-packed 2 k-tiles at a time -----
            po_a = ps.tile([P, 2, QF], F32, tag="ps")
            po_b = ps.tile([P, 2, QF], F32, tag="ps")
            if True:
                npair = NK // 2
                for p_i in range(npair):
                    ktA, ktB = 2 * p_i, 2 * p_i + 1
                    for qc in range(NQC):
                        qsl = slice(qc * QF, (qc + 1) * QF)
                        pot = po_a if qc < 2 else po_b
                        mA = nc.tensor.matmul(
                            pot[0:D, qc % 2, :],
                            vs_sb[:, ktA, :],
                            e_sb[:, ktA, qsl],
                            start=(p_i == 0), stop=(p_i == npair - 1),
                            tile_position=(0, 0), skip_group_check=True,
                        )
                        mB = nc.tensor.matmul(
                            pot[64:64 + D, qc % 2, :],
                            vs_sb[:, ktB, :],
                            e_sb[:, ktB, qsl],
                            start=(p_i == 0), stop=(p_i == npair - 1),
                            tile_position=(0, 64), skip_group_check=True,
                        )
                        if p_i == 0 and qc == 0:
                            tile.add_dep_helper(
                                last_exp.ins, mA.ins, sync=False,
                                reason="attn after last exp",
                            )
                            tile.add_dep_helper(
                                last_exp.ins, mB.ins, sync=False,
                                reason="attn after last exp",
                            )
                out3 = out_sb.rearrange("d (c f) -> d c f", f=QF)
                nc.vector.tensor_copy(out=out3[:, 0:2, :], in_=po_a[0:D, :, :])
                nc.vector.tensor_copy(out=out3[:, 2:4, :], in_=po_b[0:D, :, :])
                nc.vector.tensor_add(
                    out=out3[:, 0:2, :], in0=out3[:, 0:2, :],
                    in1=po_a[64:64 + D, :, :],
                )
                nc.vector.tensor_add(
                    out=out3[:, 2:4, :], in0=out3[:, 2:4, :],
                    in1=po_b[64:64 + D, :, :],
                )
                nc.sync.dma_start(out=out_d[:, :], in_=out_sb)

    nc.compile()
    return nc


_NC_CACHE = None


def _get_nc():
    global _NC_CACHE
    if _NC_CACHE is None:
        _NC_CACHE = _build()
    return _NC_CACHE


def kernel(x, Wq, bq, Wk, bk, Wv, bv):
    x = np.asarray(x, np.float32)
    bf = ml_dtypes.bfloat16
    wkv = np.zeros((DIM, 112), np.float32)
    wkv[:, 0:D] = np.asarray(Wk, np.float32)
    wkv[:, 64:64 + D] = np.asarray(Wv, np.float32)
    bkv = np.zeros((112,), np.float32)
    bkv[0:D] = np.asarray(bk, np.float32).ravel()
    bkv[64:64 + D] = np.asarray(bv, np.float32).ravel()
    w_bf = {
        "wq": np.ascontiguousarray(np.asarray(Wq, np.float32)).astype(bf),
        "wkv": np.ascontiguousarray(wkv).astype(bf),
    }
    b_f32 = {
        "bq": np.ascontiguousarray(np.asarray(bq, np.float32)).reshape(D, 1),
        "bkv": np.ascontiguousarray(bkv).reshape(112, 1),
    }

    in_maps = []
    for core in range(N_CORES):
        b_idx, h = divmod(core, 2)
        xt = np.ascontiguousarray(x[b_idx].T).astype(bf)          # [768, 4096]
        xtq = np.ascontiguousarray(xt[:, h * SH:(h + 1) * SH])    # [768, 2048]
        in_maps.append({"xt": xt, "xtq": xtq, **w_bf, **b_f32})

    res = run_bass_kernel_spmd(
        _get_nc(), in_maps, core_ids=list(range(N_CORES)), trace=False
    )

    out = np.empty((B, S, D), np.float32)
    for core in range(N_CORES):
        b_idx, h = divmod(core, 2)
        out[b_idx, h * SH:(h + 1) * SH, :] = res.results[core]["out"].T
    return out
